# revision 8
# baseline (speedup 1.0000x reference)
"""AggregationLoss Trainium2 kernel (nn_AggregationLoss_19258633355266).

Reference math: per sample b and instance i in 1..8, over per-pixel channel
energy s = sum_c pred[b,c,:]^2 and instance-id maps t (text) and k (kernel):
    ct_i = #{t==i}, ck_i = #{k==i}
    A_i  = sum s[t==i], Bk_i = sum s[k==i], D_i = sum s[(t==k)&(k==i)]
    ss   = A + Bk/ck^2 - 2 D/ck ; loss_i = log1p((sqrt(ss)-0.5)^2)/ct
summed over valid segments (ct>0, ck>0, ss>0, i>=1).

Distribution: data-parallel over batch B=16 across 8 NeuronCores, 2 samples
per core packed along the partition axis (64 rows each, free dim W=6400).

Device architecture (per core, ~45 compute instructions, all bf16):
  Every reduction is a fused full-width pass with an f32 accumulator,
  balanced across the two throughput engines:
  - DVE scalar_tensor_tensor (1x, accum verified-correct on HW):
      Bk_i = acc[(k==i)*s],  D_i = acc[(code2==i)*s], code2 = t*(t==k)
  - ACT activation + accum (1x, runs fully in parallel with DVE):
      sgn_i = acc[Sign(lab - i + .5)]    -> exact count ladders N_{lab>=i}
      bt_i  = acc[Relu(v_t - 64 i)]      -> A-band sums, v_t = 64 t + s
  - squares run on DVE (tensor_tensor mult, 2x); the channel sum and
    v_t build are 2x tensor_tensor adds reusing pred's SBUF slices.
Host recovery (float64, exact algebra):
  ct_i = N_ge_i - N_ge_{i+1}         (sign ladders; exact integers)
  A_i  = bt_i - bt_{i+1} - 64 * N_ge_{i+1}(t)
Host->device payload: pred as bf16 (B, PS, C, W), labels packed (B, PS, 2, W)
so each tensor loads with one descriptor-efficient DMA per channel/map.
"""

import sys

import numpy as np

import ml_dtypes

B = 16
C = 4
NPIX = 640 * 640
P = 128
PS = 64                    # partitions per sample
W = NPIX // PS             # 6400 free-dim elements per sample row
B_LOC = 2                  # samples per core
N_CORES = 8
NI = 8                     # instances 1..8 (0 = background, always invalid)
K = 64.0                   # A-band width; needs max(s) < 64 (chi2(4) max ~45)
SIGMA = 0.5

# stats layout (f32, per partition):
#  0..7 sgn_t   8..15 sgn_k   16..23 D   24..31 Bk   40..47 bt (A-bands)
NSTAT = 48

_NC = None


def _import_concourse():
    try:
        import concourse.bacc  # noqa: F401
    except ImportError:
        sys.path.append("/opt/trn_rl_repo")
        import concourse.bacc  # noqa: F401


def _build_nc(repeat=1, wlist=None):
    _import_concourse()
    import concourse.bacc as bacc
    import concourse.mybir as mybir
    import concourse.tile as tile
    from contextlib import ExitStack

    f32 = mybir.dt.float32
    bf16 = mybir.dt.bfloat16
    eq = mybir.AluOpType.is_equal
    add = mybir.AluOpType.add
    mult = mybir.AluOpType.mult
    RELU = mybir.ActivationFunctionType.Relu
    SIGN = mybir.ActivationFunctionType.Sign

    nc = bacc.Bacc("TRN2", target_bir_lowering=False, debug=False,
                   num_devices=N_CORES)
    pred_d = nc.declare_dram_parameter("pred", [B_LOC, PS, C, W], bf16,
                                       isOutput=False)
    lab_d = nc.declare_dram_parameter("labs", [B_LOC, PS, 2, W], bf16,
                                      isOutput=False)
    stats_d = nc.declare_dram_parameter("stats", [P, NSTAT], f32, isOutput=True)

    with tile.TileContext(nc) as tc, ExitStack() as ctx:
        cpool = ctx.enter_context(tc.tile_pool(name="c", bufs=1))
        opool = ctx.enter_context(tc.tile_pool(name="o", bufs=2))
        lpool = ctx.enter_context(tc.tile_pool(name="l", bufs=2))
        apool = ctx.enter_context(tc.tile_pool(name="a", bufs=1))

        # ACT bias constants (per-partition APs), hoisted out of the loop
        biasv = cpool.tile([P, 2 * NI], f32, tag="biasv")
        for i in range(1, NI + 1):
            nc.vector.memset(biasv[:, i - 1:i], -(i - 0.5))
            nc.vector.memset(biasv[:, NI + i - 1:NI + i], -K * i)

        for _rep in range(repeat):
            w = W if wlist is None else wlist[_rep]
            L = lpool.tile([P, 2, W], bf16, tag="L")
            predt = cpool.tile([P, C, W], bf16, tag="predt")

            def lab_dma(m):
                nc.sync.dma_start(
                    L[:, m, 0:w],
                    lab_d[:, :, m, 0:w].rearrange("b p w -> (b p) w"))

            def pred_dma(c):
                nc.sync.dma_start(
                    predt[:, c, 0:w],
                    pred_d[:, :, c, 0:w].rearrange("b p w -> (b p) w"))

            # labels first (unblock ACT signs + DVE label prep), then pred
            lab_dma(0)
            lab_dma(1)
            for c in range(C):
                pred_dma(c)
            t = L[:, 0, 0:w]
            k = L[:, 1, 0:w]

            stats = cpool.tile([P, NSTAT], f32, tag="stats")

            def stt(in0, scalar, in1, col):
                out = opool.tile([P, W], bf16, tag="out")
                nc.vector.scalar_tensor_tensor(
                    out=out[:, 0:w], in0=in0, scalar=float(scalar), in1=in1,
                    op0=eq, op1=mult, accum_out=stats[:, col:col + 1])

            def act_pass(v, func, bias_col, col):
                out = apool.tile([P, W], bf16, tag="aout")
                nc.scalar.activation(
                    out[:, 0:w], v, func, bias=biasv[:, bias_col:bias_col + 1],
                    scale=1.0, accum_out=stats[:, col:col + 1])

            # squares in place on DVE (tensor_tensor mult is 2x)
            for c in range(C):
                nc.vector.tensor_tensor(out=predt[:, c, 0:w],
                                        in0=predt[:, c, 0:w],
                                        in1=predt[:, c, 0:w], op=mult)

            # label prep on DVE
            mtk = cpool.tile([P, W], bf16, tag="mtk")
            code2 = cpool.tile([P, W], bf16, tag="code2")
            nc.vector.tensor_tensor(out=mtk[:, 0:w], in0=t, in1=k, op=eq)
            nc.vector.tensor_tensor(out=code2[:, 0:w], in0=mtk[:, 0:w], in1=t,
                                    op=mult)
            t64 = cpool.tile([P, W], bf16, tag="t64")
            nc.vector.tensor_scalar(out=t64[:, 0:w], in0=t, scalar1=K,
                                    scalar2=None, op0=mult)

            # s chain: partials in predt slices, s and v_t in their own
            # tiles so predt frees mid-rep and the next rep's pred DMA
            # can prefetch during this rep's accumulation passes
            s01 = predt[:, 0, 0:w]
            s23 = predt[:, 2, 0:w]
            s_t = cpool.tile([P, W], bf16, tag="s_t")
            vt_t = cpool.tile([P, W], bf16, tag="vt_t")
            s = s_t[:, 0:w]
            nc.vector.tensor_tensor(out=s01, in0=predt[:, 0, 0:w],
                                    in1=predt[:, 1, 0:w], op=add)
            nc.vector.tensor_tensor(out=s23, in0=predt[:, 2, 0:w],
                                    in1=predt[:, 3, 0:w], op=add)
            nc.vector.tensor_tensor(out=s, in0=s01, in1=s23, op=add)
            v_t = vt_t[:, 0:w]
            nc.vector.tensor_tensor(out=v_t, in0=s, in1=t64[:, 0:w], op=add)

            # ACT passes: count ladders for both maps, A-bands on v_t
            for i in range(1, NI + 1):
                act_pass(t, SIGN, i - 1, i - 1)
            for i in range(1, NI + 1):
                act_pass(k, SIGN, i - 1, 8 + i - 1)
            for i in range(1, NI + 1):
                act_pass(v_t, RELU, NI + i - 1, 40 + i - 1)

            # DVE direct stats
            for i in range(1, NI + 1):
                stt(code2[:, 0:w], i, s, 16 + i - 1)     # D_i
            for i in range(1, NI + 1):
                stt(k, i, s, 24 + i - 1)                 # Bk_i

            nc.sync.dma_start(stats_d[:], stats[:])
    nc.finalize()
    return nc


def _get_nc():
    global _NC
    if _NC is None:
        _NC = _build_nc()
    return _NC


def make_in_maps(pred, tlab, klab):
    """Host-side sharding: bf16 cast + per-core slices (pred p-major)."""
    pred = np.asarray(pred).astype(ml_dtypes.bfloat16).reshape(B, C, PS, W)
    pred = np.ascontiguousarray(pred.transpose(0, 2, 1, 3))  # (B, PS, C, W)
    tlab = np.asarray(tlab).astype(ml_dtypes.bfloat16).reshape(B, PS, W)
    klab = np.asarray(klab).astype(ml_dtypes.bfloat16).reshape(B, PS, W)
    labs = np.stack([tlab, klab], axis=2)                    # (B, PS, 2, W)
    in_maps = []
    for r in range(N_CORES):
        lo, hi = r * B_LOC, (r + 1) * B_LOC
        in_maps.append({"pred": pred[lo:hi], "labs": labs[lo:hi]})
    return in_maps


def run_device(pred, tlab, klab, **spmd_kwargs):
    _import_concourse()
    from concourse.bass_utils import run_bass_kernel_spmd

    nc = _get_nc()
    in_maps = make_in_maps(pred, tlab, klab)
    res = run_bass_kernel_spmd(nc, in_maps, list(range(N_CORES)), **spmd_kwargs)
    raw = np.zeros((B, NSTAT), np.float64)
    for r in range(N_CORES):
        out = np.asarray(res.results[r]["stats"], dtype=np.float64)
        for b in range(B_LOC):
            raw[r * B_LOC + b] = out[b * PS:(b + 1) * PS].sum(axis=0)
    return raw, res


def recover(raw):
    """(B, 48) raw device sums -> per-sample ct, ck, A, Bk, D (float64)."""
    npix = float(PS * W)

    def counts_from_signs(sgn):
        n_ge = (sgn + npix) / 2.0
        n_ge_next = np.concatenate([n_ge[:, 1:], np.zeros_like(n_ge[:, :1])],
                                   axis=1)
        return n_ge - n_ge_next, n_ge_next

    ct, n_gt_t = counts_from_signs(raw[:, 0:8])
    ck, _ = counts_from_signs(raw[:, 8:16])
    D = raw[:, 16:24]
    Bk = raw[:, 24:32]
    bt = raw[:, 40:48]
    bt_next = np.concatenate([bt[:, 1:], np.zeros_like(bt[:, :1])], axis=1)
    A = bt - bt_next - K * n_gt_t
    return ct, ck, A, Bk, D


def finish(raw):
    """Final scalar loss from raw device stats (float64 on host)."""
    ct, ck, A, Bk, D = recover(raw)
    kc = np.where(ck > 0, ck, 1.0)
    tcs = np.where(ct > 0, ct, 1.0)
    ss = A + Bk / (kc * kc) - 2.0 * D / kc
    ss_safe = np.where(ss > 0, ss, 1.0)
    norm = np.sqrt(ss_safe) - SIGMA
    loss = np.log1p(norm * norm) / tcs
    valid = (ct > 0) & (ck > 0) & (ss > 0)
    return np.array(np.sum(np.where(valid, loss, 0.0)), dtype=np.float32)


def kernel(pred_similarities, regions_mask=None, kernels_mask=None,
           text_mask_ndi_labels=None, kernel_mask_ndi_labels=None):
    raw, _ = run_device(pred_similarities, text_mask_ndi_labels,
                        kernel_mask_ndi_labels)
    return finish(raw)


# revision 10
# speedup vs baseline: 1.0863x; 1.0863x over previous
"""AggregationLoss Trainium2 kernel (nn_AggregationLoss_19258633355266).

Reference math: per sample b and instance i in 1..8, over per-pixel channel
energy s = sum_c pred[b,c,:]^2 and instance-id maps t (text) and k (kernel):
    ct_i = #{t==i}, ck_i = #{k==i}
    A_i  = sum s[t==i], Bk_i = sum s[k==i], D_i = sum s[(t==k)&(k==i)]
    ss   = A + Bk/ck^2 - 2 D/ck ; loss_i = log1p((sqrt(ss)-0.5)^2)/ct
summed over valid segments (ct>0, ck>0, ss>0, i>=1).

Distribution: data-parallel over batch B=16 across 8 NeuronCores, 2 samples
per core packed along the partition axis (64 rows each, free dim W=6400).

Device architecture (per core, ~45 compute instructions, all bf16):
  Every reduction is a fused full-width pass with an f32 accumulator,
  balanced across the two throughput engines:
  - DVE scalar_tensor_tensor (1x, accum verified-correct on HW):
      Bk_i = acc[(k==i)*s],  D_i = acc[(code2==i)*s], code2 = t*(t==k)
  - ACT activation + accum (1x, runs fully in parallel with DVE):
      sgn_i = acc[Sign(lab - i + .5)]    -> exact count ladders N_{lab>=i}
      bt_i  = acc[Relu(v_t - 64 i)]      -> A-band sums, v_t = 64 t + s
  - squares run on DVE (tensor_tensor mult, 2x); the channel sum and
    v_t build are 2x tensor_tensor adds reusing pred's SBUF slices.
Host recovery (float64, exact algebra):
  ct_i = N_ge_i - N_ge_{i+1}         (sign ladders; exact integers)
  A_i  = bt_i - bt_{i+1} - 64 * N_ge_{i+1}(t)
Host->device payload: pred as bf16 (B, PS, C, W), labels packed (B, PS, 2, W)
so each tensor loads with one descriptor-efficient DMA per channel/map.
"""

import sys

import numpy as np

import ml_dtypes

B = 16
C = 4
NPIX = 640 * 640
P = 128
PS = 64                    # partitions per sample
W = NPIX // PS             # 6400 free-dim elements per sample row
B_LOC = 2                  # samples per core
N_CORES = 8
NI = 8                     # instances 1..8 (0 = background, always invalid)
K = 64.0                   # A-band width; needs max(s) < 64 (chi2(4) max ~45)
SIGMA = 0.5

# stats layout (f32, per partition):
#  0..7 sgn_t   8..15 sgn_k   16..23 D   24..31 Bk   40..47 bt (A-bands)
NSTAT = 48

_NC = None


def _import_concourse():
    try:
        import concourse.bacc  # noqa: F401
    except ImportError:
        sys.path.append("/opt/trn_rl_repo")
        import concourse.bacc  # noqa: F401


def _build_nc(repeat=1, wlist=None):
    _import_concourse()
    import concourse.bacc as bacc
    import concourse.mybir as mybir
    import concourse.tile as tile
    from contextlib import ExitStack

    f32 = mybir.dt.float32
    bf16 = mybir.dt.bfloat16
    eq = mybir.AluOpType.is_equal
    add = mybir.AluOpType.add
    mult = mybir.AluOpType.mult
    RELU = mybir.ActivationFunctionType.Relu
    SIGN = mybir.ActivationFunctionType.Sign

    nc = bacc.Bacc("TRN2", target_bir_lowering=False, debug=False,
                   num_devices=N_CORES)
    pred_d = nc.declare_dram_parameter("pred", [B_LOC, PS, C, W], bf16,
                                       isOutput=False)
    lab_d = nc.declare_dram_parameter("labs", [B_LOC, PS, 2, W], bf16,
                                      isOutput=False)
    stats_d = nc.declare_dram_parameter("stats", [P, NSTAT], f32, isOutput=True)

    with tile.TileContext(nc) as tc, ExitStack() as ctx:
        cpool = ctx.enter_context(tc.tile_pool(name="c", bufs=1))
        opool = ctx.enter_context(tc.tile_pool(name="o", bufs=2))
        lpool = ctx.enter_context(tc.tile_pool(name="l", bufs=2))
        apool = ctx.enter_context(tc.tile_pool(name="a", bufs=1))

        # ACT bias constants (per-partition APs), hoisted out of the loop
        biasv = cpool.tile([P, 2 * NI], f32, tag="biasv")
        for i in range(1, NI + 1):
            nc.vector.memset(biasv[:, i - 1:i], -(i - 0.5))
            nc.vector.memset(biasv[:, NI + i - 1:NI + i], -K * i)

        for _rep in range(repeat):
            w = W if wlist is None else wlist[_rep]
            L = lpool.tile([P, 2, W], bf16, tag="L")
            predt = cpool.tile([P, C, W], bf16, tag="predt")

            def lab_dma(m):
                nc.sync.dma_start(
                    L[:, m, 0:w],
                    lab_d[:, :, m, 0:w].rearrange("b p w -> (b p) w"))

            def pred_dma(c):
                nc.sync.dma_start(
                    predt[:, c, 0:w],
                    pred_d[:, :, c, 0:w].rearrange("b p w -> (b p) w"))

            # labels first (unblock ACT signs + DVE label prep), then pred
            lab_dma(0)
            lab_dma(1)
            for c in range(C):
                pred_dma(c)
            t = L[:, 0, 0:w]
            k = L[:, 1, 0:w]

            stats = cpool.tile([P, NSTAT], f32, tag="stats")

            def stt(in0, scalar, in1, col):
                out = opool.tile([P, W], bf16, tag="out")
                nc.vector.scalar_tensor_tensor(
                    out=out[:, 0:w], in0=in0, scalar=float(scalar), in1=in1,
                    op0=eq, op1=mult, accum_out=stats[:, col:col + 1])

            def act_pass(v, func, bias_col, col):
                out = apool.tile([P, W], bf16, tag="aout")
                nc.scalar.activation(
                    out[:, 0:w], v, func, bias=biasv[:, bias_col:bias_col + 1],
                    scale=1.0, accum_out=stats[:, col:col + 1])

            # squares in place on DVE (tensor_tensor mult is 2x)
            for c in range(C):
                nc.vector.tensor_tensor(out=predt[:, c, 0:w],
                                        in0=predt[:, c, 0:w],
                                        in1=predt[:, c, 0:w], op=mult)

            # label prep on DVE
            mtk = cpool.tile([P, W], bf16, tag="mtk")
            code2 = cpool.tile([P, W], bf16, tag="code2")
            nc.vector.tensor_tensor(out=mtk[:, 0:w], in0=t, in1=k, op=eq)
            nc.vector.tensor_tensor(out=code2[:, 0:w], in0=mtk[:, 0:w], in1=t,
                                    op=mult)
            t64 = cpool.tile([P, W], bf16, tag="t64")
            nc.vector.tensor_scalar(out=t64[:, 0:w], in0=t, scalar1=K,
                                    scalar2=None, op0=mult)

            # s chain: partials in predt slices, s and v_t in their own
            # tiles so predt frees mid-rep and the next rep's pred DMA
            # can prefetch during this rep's accumulation passes
            s01 = predt[:, 0, 0:w]
            s23 = predt[:, 2, 0:w]
            s_t = cpool.tile([P, W], bf16, tag="s_t")
            vt_t = cpool.tile([P, W], bf16, tag="vt_t")
            s = s_t[:, 0:w]
            nc.vector.tensor_tensor(out=s01, in0=predt[:, 0, 0:w],
                                    in1=predt[:, 1, 0:w], op=add)
            nc.vector.tensor_tensor(out=s23, in0=predt[:, 2, 0:w],
                                    in1=predt[:, 3, 0:w], op=add)
            nc.vector.tensor_tensor(out=s, in0=s01, in1=s23, op=add)
            v_t = vt_t[:, 0:w]
            nc.vector.tensor_tensor(out=v_t, in0=s, in1=t64[:, 0:w], op=add)

            # ACT passes: count ladders for both maps, A-bands on v_t
            for i in range(1, NI + 1):
                act_pass(t, SIGN, i - 1, i - 1)
            for i in range(1, NI + 1):
                act_pass(k, SIGN, i - 1, 8 + i - 1)
            for i in range(1, NI + 1):
                act_pass(v_t, RELU, NI + i - 1, 40 + i - 1)

            # DVE direct stats
            for i in range(1, NI + 1):
                stt(code2[:, 0:w], i, s, 16 + i - 1)     # D_i
            for i in range(1, NI + 1):
                stt(k, i, s, 24 + i - 1)                 # Bk_i

            nc.sync.dma_start(stats_d[:], stats[:])
    nc.finalize()
    return nc


def _get_nc():
    global _NC
    if _NC is None:
        _NC = _build_nc()
    return _NC


def make_in_maps(pred, tlab, klab):
    """Host-side sharding: bf16 cast + per-core slices (pred p-major)."""
    pred = np.asarray(pred).astype(ml_dtypes.bfloat16).reshape(B, C, PS, W)
    pred = np.ascontiguousarray(pred.transpose(0, 2, 1, 3))  # (B, PS, C, W)
    tlab = np.asarray(tlab).astype(ml_dtypes.bfloat16).reshape(B, PS, W)
    klab = np.asarray(klab).astype(ml_dtypes.bfloat16).reshape(B, PS, W)
    labs = np.stack([tlab, klab], axis=2)                    # (B, PS, 2, W)
    in_maps = []
    for r in range(N_CORES):
        lo, hi = r * B_LOC, (r + 1) * B_LOC
        in_maps.append({"pred": pred[lo:hi], "labs": labs[lo:hi]})
    return in_maps


def run_device(pred, tlab, klab, **spmd_kwargs):
    _import_concourse()
    from concourse.bass_utils import run_bass_kernel_spmd

    nc = _get_nc()
    in_maps = make_in_maps(pred, tlab, klab)
    res = run_bass_kernel_spmd(nc, in_maps, list(range(N_CORES)), **spmd_kwargs)
    raw = np.zeros((B, NSTAT), np.float64)
    for r in range(N_CORES):
        out = np.asarray(res.results[r]["stats"], dtype=np.float64)
        for b in range(B_LOC):
            raw[r * B_LOC + b] = out[b * PS:(b + 1) * PS].sum(axis=0)
    return raw, res


def recover(raw):
    """(B, 48) raw device sums -> per-sample ct, ck, A, Bk, D (float64)."""
    npix = float(PS * W)

    def counts_from_signs(sgn):
        n_ge = (sgn + npix) / 2.0
        n_ge_next = np.concatenate([n_ge[:, 1:], np.zeros_like(n_ge[:, :1])],
                                   axis=1)
        return n_ge - n_ge_next, n_ge_next

    ct, n_gt_t = counts_from_signs(raw[:, 0:8])
    ck, _ = counts_from_signs(raw[:, 8:16])
    D = raw[:, 16:24]
    Bk = raw[:, 24:32]
    bt = raw[:, 40:48]
    bt_next = np.concatenate([bt[:, 1:], np.zeros_like(bt[:, :1])], axis=1)
    A = bt - bt_next - K * n_gt_t
    return ct, ck, A, Bk, D


def finish(raw):
    """Final scalar loss from raw device stats (float64 on host)."""
    ct, ck, A, Bk, D = recover(raw)
    kc = np.where(ck > 0, ck, 1.0)
    tcs = np.where(ct > 0, ct, 1.0)
    ss = A + Bk / (kc * kc) - 2.0 * D / kc
    ss_safe = np.where(ss > 0, ss, 1.0)
    norm = np.sqrt(ss_safe) - SIGMA
    loss = np.log1p(norm * norm) / tcs
    valid = (ct > 0) & (ck > 0) & (ss > 0)
    return np.array(np.sum(np.where(valid, loss, 0.0)), dtype=np.float32)


def kernel(pred_similarities, regions_mask=None, kernels_mask=None,
           text_mask_ndi_labels=None, kernel_mask_ndi_labels=None):
    raw, _ = run_device(pred_similarities, text_mask_ndi_labels,
                        kernel_mask_ndi_labels)
    return finish(raw)
